# revision 1
# baseline (speedup 1.0000x reference)
"""Trainium2 kernel for nn_Experts (MoE grouped expert GEMM).

Problem: input [16384, 2048] f32, weight [8, 8192, 2048] f32, bias [8, 8192]
f32, expert_frequency [8] int32 (balanced: 2048 tokens/expert, pre-grouped),
capacity 2048.  Output [16384, 8192] f32 with out[t] = W_e x[t] + b_e.

Sharding: expert parallelism — core e computes expert e's GEMM
  Y_e = X_e @ W_e^T + b_e   (X_e [2048, 2048], W_e [8192, 2048])

Per-core kernel computes YT_e = W_e X_e^T + b_e  ([OUT, TOK], transposed
output; the host transposes back).  Matmul precision: split-precision
bf16x3 — x = xh + xl, w = wh + wl (bf16 halves), y = xh@wh + xh@wl + xl@wh,
which carries ~1e-5 relative error (fp32-grade) at 3x bf16 throughput
(native fp32 on the PE costs 4x bf16; broken fp32r would be 1x).

Raw Bass (no Tile): the walrus build here rejects any engine instruction
with more than one sync wait, so all cross-engine sync is explicit
single-semaphore waits:
  SP   : input DMAs (X slices, W tiles, bias) + W-slot-reuse waits
  PE   : 12288 matmuls (64 out-tiles x 4 tok-slices x 16 k-chunks x 3 terms)
  DVE  : PSUM -> SBUF eviction fused with per-partition bias add
  ACT  : output DMAs
"""

import numpy as np

import concourse.bass as bass
import concourse.mybir as mybir
from concourse.bass_utils import run_bass_kernel_spmd

# problem shape (per core)
E = 8
TOK = 2048      # tokens per expert (= capacity)
IN = 2048       # in features (contraction)
OUT = 8192      # out features
T_FULL = E * TOK

KC = IN // 128          # 16 contraction chunks
SLICE = 512             # moving-dim (token) slice
TS = TOK // SLICE       # 4 token slices
OT = OUT // 128         # 64 out tiles
G = OT * TS             # 256 groups
NPSUM = 4               # psum/y slot rotation
NW = 2                  # w slot rotation (double buffer)

F32 = mybir.dt.float32

# MODE: 'bf16x3' (default, fp32-grade), 'bf16', 'fp16', 'fp32'
_MODES = {
    # mode: (mm dtype, n_terms)
    "bf16x3": (mybir.dt.bfloat16, 3),
    "bf16": (mybir.dt.bfloat16, 1),
    "fp16": (mybir.dt.float16, 1),
    "fp32": (mybir.dt.float32, 1),
}
MODE = "bf16x3"


def build(mode: str = MODE) -> bass.Bass:
    mm_dt, n_terms = _MODES[mode]
    split = n_terms == 3

    nc = bass.Bass(target_bir_lowering=False)
    xh = nc.dram_tensor("xh", [IN, TOK], mm_dt, kind="ExternalInput")
    wh = nc.dram_tensor("wh", [IN, OUT], mm_dt, kind="ExternalInput")
    if split:
        xl = nc.dram_tensor("xl", [IN, TOK], mm_dt, kind="ExternalInput")
        wl = nc.dram_tensor("wl", [IN, OUT], mm_dt, kind="ExternalInput")
    bias = nc.dram_tensor("bias", [128, OT], F32, kind="ExternalInput")
    yt = nc.dram_tensor("yt", [OUT, TOK], F32, kind="ExternalOutput")

    # [128, KC, *] views with chunk c covering rows c*128 .. c*128+127
    xh_r = xh[:, :].rearrange("(c p) t -> p c t", p=128)
    wh_r = wh[:, :].rearrange("(c p) o -> p c o", p=128)
    if split:
        xl_r = xl[:, :].rearrange("(c p) t -> p c t", p=128)
        wl_r = wl[:, :].rearrange("(c p) o -> p c o", p=128)

    nhalf = 2 if split else 1

    with (
        nc.sbuf_tensor("x_sb", [128, nhalf, KC, TOK], mm_dt) as x_sb,
        nc.sbuf_tensor("w_sb", [128, NW, nhalf, KC, 128], mm_dt) as w_sb,
        nc.sbuf_tensor("y_sb", [128, NPSUM, SLICE], F32) as y_sb,
        nc.sbuf_tensor("b_sb", [128, OT], F32) as b_sb,
        nc.psum_tensor("acc", [128, NPSUM, SLICE], F32) as acc,
        nc.semaphore("sem_x") as sem_x,
        nc.semaphore("sem_w") as sem_w,
        nc.semaphore("sem_pe") as sem_pe,
        nc.semaphore("sem_dve") as sem_dve,
        nc.semaphore("sem_dout") as sem_dout,
        nc.Block() as block,
    ):
        # sem_x increments (x16): per t: X halves; bias right after t=0.
        # x_done[t] = sem_x value after which X slice t (all halves) is loaded
        x_done = []
        bias_done = None

        @block.sync
        def _(sp):
            nonlocal bias_done
            v = 0
            for t in range(TS):
                sl = slice(t * SLICE, (t + 1) * SLICE)
                sp.dma_start(x_sb[:, 0, :, sl], xh_r[:, :, sl]).then_inc(sem_x, 16)
                v += 16
                if split:
                    sp.dma_start(x_sb[:, 1, :, sl], xl_r[:, :, sl]).then_inc(sem_x, 16)
                    v += 16
                x_done.append(v)
                if t == 0:
                    sp.dma_start(b_sb[:], bias[:]).then_inc(sem_x, 16)
                    v += 16
                    bias_done = v
            for o in range(OT):
                if o >= NW:
                    # PE done reading w slot o-NW after its last group:
                    # sem_pe >= (o-NW+1)*TS
                    sp.wait_ge(sem_pe, (o - NW + 1) * TS)
                osl = slice(o * 128, (o + 1) * 128)
                sp.dma_start(w_sb[:, o % NW, 0, :, :], wh_r[:, :, osl]).then_inc(
                    sem_w, 16
                )
                if split:
                    sp.dma_start(w_sb[:, o % NW, 1, :, :], wl_r[:, :, osl]).then_inc(
                        sem_w, 16
                    )
            # all output DMAs complete before NEFF completion
            sp.wait_ge(sem_dout, 16 * G)

        w_per_o = 16 * nhalf

        @block.tensor
        def _(pe):
            g = 0
            for o in range(OT):
                pe.wait_ge(sem_w, w_per_o * (o + 1))
                for t in range(TS):
                    if o == 0:
                        pe.wait_ge(sem_x, x_done[t])
                    if g >= NPSUM:
                        pe.wait_ge(sem_dve, g - NPSUM + 1)
                    s = g % NPSUM
                    xsl = slice(t * SLICE, (t + 1) * SLICE)
                    # accumulation group: 16 k-chunks x n_terms matmuls
                    n_mm = KC * n_terms
                    i = 0
                    for k in range(KC):
                        # terms: (wh,xh), (wl,xh), (wh,xl)
                        terms = [(0, 0)] if not split else [(0, 0), (1, 0), (0, 1)]
                        for (w_i, x_i) in terms:
                            mm = pe.matmul(
                                acc[:, s, :],
                                w_sb[:, o % NW, w_i, k, :],
                                x_sb[:, x_i, k, xsl],
                                start=(i == 0),
                                stop=(i == n_mm - 1),
                            )
                            i += 1
                    mm.then_inc(sem_pe, 1)
                    g += 1

        @block.vector
        def _(dve):
            for g in range(G):
                o = g // TS
                dve.wait_ge(sem_pe, g + 1)
                if g == 0:
                    dve.wait_ge(sem_x, bias_done)
                if g >= NPSUM:
                    dve.wait_ge(sem_dout, 16 * (g - NPSUM + 1))
                s = g % NPSUM
                dve.tensor_scalar_add(
                    y_sb[:, s, :], acc[:, s, :], b_sb[:, o:o + 1]
                ).then_inc(sem_dve, 1)

        @block.scalar
        def _(act):
            for g in range(G):
                o, t = divmod(g, TS)
                act.wait_ge(sem_dve, g + 1)
                s = g % NPSUM
                act.dma_start(
                    yt[o * 128:(o + 1) * 128, t * SLICE:(t + 1) * SLICE],
                    y_sb[:, s, :],
                ).then_inc(sem_dout, 16)

    return nc


_nc_cache: dict = {}


def _get_nc(mode: str) -> bass.Bass:
    if mode not in _nc_cache:
        _nc_cache[mode] = build(mode)
    return _nc_cache[mode]


def _make_in_maps(input, weight, bias, expert_frequency, mode: str):
    mm_dt, n_terms = _MODES[mode]
    np_dt = mybir.dt.np(mm_dt)
    split = n_terms == 3

    freq = np.asarray(expert_frequency, dtype=np.int64)
    ends = np.cumsum(freq)
    starts = ends - freq

    input = np.asarray(input, dtype=np.float32)
    weight = np.asarray(weight, dtype=np.float32)
    bias = np.asarray(bias, dtype=np.float32)

    in_maps = []
    for e in range(E):
        n = int(min(freq[e], TOK))
        x = np.zeros((TOK, IN), dtype=np.float32)
        x[:n] = input[starts[e]:starts[e] + n]
        xt = np.ascontiguousarray(x.T)                       # [IN, TOK]
        wt = np.ascontiguousarray(weight[e].T)               # [IN, OUT]
        br = np.ascontiguousarray(bias[e].reshape(OT, 128).T)  # [128, OT]

        xh = xt.astype(np_dt)
        wh = wt.astype(np_dt)
        m = {"xh": xh, "wh": wh, "bias": br}
        if split:
            m["xl"] = (xt - xh.astype(np.float32)).astype(np_dt)
            m["wl"] = (wt - wh.astype(np.float32)).astype(np_dt)
        in_maps.append(m)
    return in_maps, freq, starts


def _gather_out(results, freq, starts):
    out = np.zeros((int(np.sum(freq)), OUT), dtype=np.float32)
    for e in range(E):
        n = int(min(freq[e], TOK))
        yt = np.asarray(results[e]["yt"])    # [OUT, TOK]
        out[starts[e]:starts[e] + n] = yt[:, :n].T
    return out


def kernel(input, weight, bias, expert_frequency, capacity=None, *,
           mode: str = MODE, trace: bool = False):
    """Full-input entry point: shards per expert across 8 cores, runs the
    Bass kernel, gathers the full [T, OUT] float32 output."""
    in_maps, freq, starts = _make_in_maps(
        input, weight, bias, expert_frequency, mode
    )
    nc = _get_nc(mode)
    res = run_bass_kernel_spmd(
        nc, in_maps, core_ids=list(range(E)), trace=trace
    )
    out = _gather_out(res.results, freq, starts)
    if trace:
        return out, res
    return out


# revision 4
# speedup vs baseline: 11.7824x; 11.7824x over previous
"""Trainium2 kernel for nn_Experts (MoE grouped expert GEMM).

Problem: input [16384, 2048] f32, weight [8, 8192, 2048] f32, bias [8, 8192]
f32, expert_frequency [8] int32 (balanced: 2048 tokens/expert, pre-grouped),
capacity 2048.  Output [16384, 8192] f32 with out[t] = W_e x[t] + b_e.

Sharding: expert parallelism — core e computes expert e's GEMM
  Y_e = X_e @ W_e^T + b_e   (X_e [2048, 2048], W_e [8192, 2048])

Per-core kernel computes YT_e = W_e X_e^T + b_e  ([OUT, TOK], transposed
output; the host transposes back).  Matmul precision: split-precision
bf16x3 — x = xh + xl, w = wh + wl (bf16 halves), y = xh@wh + xh@wl + xl@wh,
which carries ~1e-5 relative error (fp32-grade) at 3x bf16 throughput
(native fp32 on the PE costs 4x bf16; broken fp32r would be 1x).

Raw Bass (no Tile): the walrus build here rejects any engine instruction
with more than one sync wait, so all cross-engine sync is explicit
single-semaphore waits:
  SP   : input DMAs (X slices, W tiles, bias) + W-slot-reuse waits
  PE   : 12288 matmuls (64 out-tiles x 4 tok-slices x 16 k-chunks x 3 terms)
  DVE  : PSUM -> SBUF eviction fused with per-partition bias add
  ACT  : output DMAs
"""

import numpy as np

import concourse.bass as bass
import concourse.mybir as mybir
from concourse.bass_utils import run_bass_kernel_spmd

# problem shape (per core)
E = 8
TOK = 2048      # tokens per expert (= capacity)
IN = 2048       # in features (contraction)
OUT = 8192      # out features
T_FULL = E * TOK

KC = IN // 128          # 16 contraction chunks
SLICE = 512             # moving-dim (token) slice
TS = TOK // SLICE       # 4 token slices
OT = OUT // 128         # 64 out tiles
G = OT * TS             # 256 groups
NPSUM = 4               # psum/y slot rotation
NW = 2                  # w slot rotation (double buffer)

F32 = mybir.dt.float32

# MODE: 'bf16x3' (default, fp32-grade), 'bf16', 'fp16', 'fp32'
_MODES = {
    # mode: (mm dtype, n_terms)
    "bf16x3": (mybir.dt.bfloat16, 3),
    "bf16": (mybir.dt.bfloat16, 1),
    "fp16": (mybir.dt.float16, 1),
    "fp32": (mybir.dt.float32, 1),
}
MODE = "bf16x3"


def build(mode: str = MODE, reps: int = 1, bench: bool = False) -> bass.Bass:
    """reps: run the whole kernel body that many times back-to-back (for
    marginal-time benchmarking).  bench: make yt an internal DRAM scratch
    and expose only a tiny marker output, so per-call host<->device
    transfer is negligible during timing."""
    mm_dt, n_terms = _MODES[mode]
    split = n_terms == 3

    nc = bass.Bass(target_bir_lowering=False)
    xh = nc.dram_tensor("xh", [IN, TOK], mm_dt, kind="ExternalInput")
    wh = nc.dram_tensor("wh", [IN, OUT], mm_dt, kind="ExternalInput")
    if split:
        xl = nc.dram_tensor("xl", [IN, TOK], mm_dt, kind="ExternalInput")
        wl = nc.dram_tensor("wl", [IN, OUT], mm_dt, kind="ExternalInput")
    bias = nc.dram_tensor("bias", [128, OT], F32, kind="ExternalInput")
    if bench:
        yt = nc.dram_tensor("yt", [OUT, TOK], F32)  # internal scratch
        marker = nc.dram_tensor("marker", [128, OT], F32, kind="ExternalOutput")
    else:
        assert reps == 1
        yt = nc.dram_tensor("yt", [OUT, TOK], F32, kind="ExternalOutput")

    # [128, KC, *] views with chunk c covering rows c*128 .. c*128+127
    xh_r = xh[:, :].rearrange("(c p) t -> p c t", p=128)
    wh_r = wh[:, :].rearrange("(c p) o -> p c o", p=128)
    if split:
        xl_r = xl[:, :].rearrange("(c p) t -> p c t", p=128)
        wl_r = wl[:, :].rearrange("(c p) o -> p c o", p=128)

    nhalf = 2 if split else 1

    with (
        nc.sbuf_tensor("x_sb", [128, nhalf, KC, TOK], mm_dt) as x_sb,
        nc.sbuf_tensor("w_sb", [128, NW, nhalf, KC, 128], mm_dt) as w_sb,
        nc.sbuf_tensor("y_sb", [128, NPSUM, SLICE], F32) as y_sb,
        nc.sbuf_tensor("b_sb", [128, OT], F32) as b_sb,
        nc.psum_tensor("acc", [128, NPSUM, SLICE], F32) as acc,
        nc.semaphore("sem_x") as sem_x,
        nc.semaphore("sem_w") as sem_w,
        nc.semaphore("sem_pe") as sem_pe,
        nc.semaphore("sem_dve") as sem_dve,
        nc.semaphore("sem_dout") as sem_dout,
        nc.Block() as block,
    ):
        # sem_x increments (x16): per t: X halves; bias right after t=0.
        # x_done[t] = sem_x value after which X slice t (all halves) is loaded
        x_done = []
        bias_done = None
        GG = reps * G       # total groups across reps
        WO = reps * OT      # total W-load steps across reps
        w_per_o = 16 * nhalf

        @block.sync
        def _(sp):
            nonlocal bias_done
            v = 0
            for t in range(TS):
                sl = slice(t * SLICE, (t + 1) * SLICE)
                sp.dma_start(x_sb[:, 0, :, sl], xh_r[:, :, sl]).then_inc(sem_x, 16)
                v += 16
                if split:
                    sp.dma_start(x_sb[:, 1, :, sl], xl_r[:, :, sl]).then_inc(sem_x, 16)
                    v += 16
                x_done.append(v)
                if t == 0:
                    sp.dma_start(b_sb[:], bias[:]).then_inc(sem_x, 16)
                    v += 16
                    bias_done = v
            for wo in range(WO):
                o = wo % OT
                if wo >= NW:
                    # PE done reading w slot wo-NW after its last group:
                    # sem_pe >= (wo-NW+1)*TS
                    sp.wait_ge(sem_pe, (wo - NW + 1) * TS)
                osl = slice(o * 128, (o + 1) * 128)
                sp.dma_start(w_sb[:, wo % NW, 0, :, :], wh_r[:, :, osl]).then_inc(
                    sem_w, 16
                )
                if split:
                    sp.dma_start(w_sb[:, wo % NW, 1, :, :], wl_r[:, :, osl]).then_inc(
                        sem_w, 16
                    )
            # all output DMAs complete before NEFF completion
            sp.wait_ge(sem_dout, 16 * GG)
            if bench:
                sp.dma_start(marker[:, :], b_sb[:]).then_inc(sem_x, 16)

        @block.tensor
        def _(pe):
            gg = 0
            for wo in range(WO):
                pe.wait_ge(sem_w, w_per_o * (wo + 1))
                for t in range(TS):
                    if wo == 0:
                        pe.wait_ge(sem_x, x_done[t])
                    if gg >= NPSUM:
                        pe.wait_ge(sem_dve, gg - NPSUM + 1)
                    s = gg % NPSUM
                    xsl = slice(t * SLICE, (t + 1) * SLICE)
                    # accumulation group: 16 k-chunks x n_terms matmuls
                    n_mm = KC * n_terms
                    i = 0
                    for k in range(KC):
                        # terms: (wh,xh), (wl,xh), (wh,xl)
                        terms = [(0, 0)] if not split else [(0, 0), (1, 0), (0, 1)]
                        for (w_i, x_i) in terms:
                            mm = pe.matmul(
                                acc[:, s, :],
                                w_sb[:, wo % NW, w_i, k, :],
                                x_sb[:, x_i, k, xsl],
                                start=(i == 0),
                                stop=(i == n_mm - 1),
                            )
                            i += 1
                    mm.then_inc(sem_pe, 1)
                    gg += 1

        @block.vector
        def _(dve):
            for gg in range(GG):
                o = (gg // TS) % OT
                dve.wait_ge(sem_pe, gg + 1)
                if gg == 0:
                    dve.wait_ge(sem_x, bias_done)
                if gg >= NPSUM:
                    dve.wait_ge(sem_dout, 16 * (gg - NPSUM + 1))
                s = gg % NPSUM
                dve.tensor_scalar_add(
                    y_sb[:, s, :], acc[:, s, :], b_sb[:, o:o + 1]
                ).then_inc(sem_dve, 1)

        @block.scalar
        def _(act):
            for gg in range(GG):
                o, t = divmod(gg % G, TS)
                act.wait_ge(sem_dve, gg + 1)
                s = gg % NPSUM
                act.dma_start(
                    yt[o * 128:(o + 1) * 128, t * SLICE:(t + 1) * SLICE],
                    y_sb[:, s, :],
                ).then_inc(sem_dout, 16)

    return nc


_nc_cache: dict = {}


def _get_nc(mode: str) -> bass.Bass:
    if mode not in _nc_cache:
        _nc_cache[mode] = build(mode)
    return _nc_cache[mode]


def _make_in_maps(input, weight, bias, expert_frequency, mode: str):
    mm_dt, n_terms = _MODES[mode]
    np_dt = mybir.dt.np(mm_dt)
    split = n_terms == 3

    freq = np.asarray(expert_frequency, dtype=np.int64)
    ends = np.cumsum(freq)
    starts = ends - freq

    input = np.asarray(input, dtype=np.float32)
    weight = np.asarray(weight, dtype=np.float32)
    bias = np.asarray(bias, dtype=np.float32)

    in_maps = []
    for e in range(E):
        n = int(min(freq[e], TOK))
        x = np.zeros((TOK, IN), dtype=np.float32)
        x[:n] = input[starts[e]:starts[e] + n]
        xt = np.ascontiguousarray(x.T)                       # [IN, TOK]
        wt = np.ascontiguousarray(weight[e].T)               # [IN, OUT]
        br = np.ascontiguousarray(bias[e].reshape(OT, 128).T)  # [128, OT]

        xh = xt.astype(np_dt)
        wh = wt.astype(np_dt)
        m = {"xh": xh, "wh": wh, "bias": br}
        if split:
            m["xl"] = (xt - xh.astype(np.float32)).astype(np_dt)
            m["wl"] = (wt - wh.astype(np.float32)).astype(np_dt)
        in_maps.append(m)
    return in_maps, freq, starts


def _gather_out(results, freq, starts):
    out = np.zeros((int(np.sum(freq)), OUT), dtype=np.float32)
    for e in range(E):
        n = int(min(freq[e], TOK))
        yt = np.asarray(results[e]["yt"])    # [OUT, TOK]
        out[starts[e]:starts[e] + n] = yt[:, :n].T
    return out


def kernel(input, weight, bias, expert_frequency, capacity=None, *,
           mode: str = MODE, trace: bool = False):
    """Full-input entry point: shards per expert across 8 cores, runs the
    Bass kernel, gathers the full [T, OUT] float32 output."""
    in_maps, freq, starts = _make_in_maps(
        input, weight, bias, expert_frequency, mode
    )
    nc = _get_nc(mode)
    res = run_bass_kernel_spmd(
        nc, in_maps, core_ids=list(range(E)), trace=trace
    )
    out = _gather_out(res.results, freq, starts)
    if trace:
        return out, res
    return out
